# revision 8
# baseline (speedup 1.0000x reference)
"""Soft decision-tree layer (depth 4, 16 leaves) on 8 trn2 NeuronCores.

Sharding: 4-way data parallel (1024-token groups) x 2-way expert parallel
(8 leaves per core, one level-3 subtree half).  Each core computes, for
its 1024 tokens t and its 8 leaves l:
  partial[t,:] = sum_l path_l(t) * (x[t] @ Wl[l]) + sum_l path_l(t)*bl[l]
Host upcasts + sums the 2 expert partials per token group.

Everything is SBUF-resident (xt 2MB + wl 16MB fp16 + acc 4MB fp32).
GEMM operands are fp16 (216ns per 128x128x512 matmul measured, LDW
hidden); fp32 accumulation in PSUM/SBUF.

Schedule (from trace analysis):
- inputs stream on BOTH hardware DGE queues (sync + scalar engines);
  xt's 8 chunks alternate queues so decisions start ~8.5us and the
  first leaf pass ~13us
- warmup matmuls run on an uninitialized tile (no DMA/memset dep) so
  the PE HAM clock-gate is warm before real work; the PE stream is
  gap-free afterwards, so HAM never re-throttles
- decisions: 8 psum chains, k-outer, paced by xt chunk arrival
- path-weighted bias sum = K=32 PE matmuls of pathT.T @ bl (replaces
  ~90us of DVE tensor ops); pathT via 2 batched PE transposes; the 4
  tiles' bias matmuls are row-tiled (tile_position) so they pack into
  the PE array concurrently
- leaf passes l-outer t-inner, psl/psr 512-col chains, DVE evicts
- output DMA'd as fp16 per half-tile (host upcasts + sums)
"""

import numpy as np

B, S, H = 2, 2048, 1024
DP, EP = 4, 2            # data-parallel x expert-parallel = 8 cores
T = (B * S) // DP        # 1024 tokens per core
LPC = 16 // EP           # 8 leaves per core
NT = T // 128            # 8 token tiles per core
KC = H // 128            # 8 contraction chunks
ND = 24                  # decision columns (22 used + 2 pad)

_prog_cache = {}


def _build_program():
    if "nc" in _prog_cache:
        return _prog_cache["nc"]

    from contextlib import ExitStack
    import concourse.bacc as bacc
    import concourse.tile as tile
    import concourse.mybir as mybir

    f32 = mybir.dt.float32
    f16 = mybir.dt.float16
    MULT = mybir.AluOpType.mult
    ADD = mybir.AluOpType.add
    SIG = mybir.ActivationFunctionType.Sigmoid

    nc = bacc.Bacc("TRN2", target_bir_lowering=False, debug=False, num_devices=8)

    xt_d = nc.dram_tensor("xt", [128, KC * T], f16, kind="ExternalInput").ap()
    wl_d = nc.dram_tensor("wl", [LPC, 128, KC * H], f16, kind="ExternalInput").ap()
    wd_d = nc.dram_tensor("wd", [128, KC * ND], f16, kind="ExternalInput").ap()
    bd_d = nc.dram_tensor("bd", [1, ND], f16, kind="ExternalInput").ap()
    bl_d = nc.dram_tensor("bl", [128, H], f16, kind="ExternalInput").ap()
    ones_d = nc.dram_tensor("ones", [1, 128], f16, kind="ExternalInput").ap()
    iden_d = nc.dram_tensor("iden", [128, 128], f32, kind="ExternalInput").ap()
    out_d = nc.dram_tensor("out", [T, H], f16, kind="ExternalOutput").ap()

    with tile.TileContext(nc) as tc, ExitStack() as ctx:
        consts = ctx.enter_context(tc.tile_pool(name="consts", bufs=1))
        xt_pool = ctx.enter_context(tc.tile_pool(name="xt", bufs=1))
        wl_pool = ctx.enter_context(tc.tile_pool(name="wl", bufs=1))
        acc_pool = ctx.enter_context(tc.tile_pool(name="acc", bufs=1))
        dec_pool = ctx.enter_context(tc.tile_pool(name="dec", bufs=2))
        out_pool = ctx.enter_context(tc.tile_pool(name="o16", bufs=4))
        ps_pool = ctx.enter_context(tc.tile_pool(name="ps", bufs=8, space="PSUM"))

        # --- PE warmup; the memset rides the GpSimd queue (idle early) so
        #     the first matmul lands right after the engine preambles and
        #     covers the DMA-queue ramp (~8.5us) ---
        warm = consts.tile([128, 512], f16, tag="warm")
        nc.gpsimd.memset(warm[:], 0.0)
        wps = ps_pool.tile([128, 512], f32, tag="ps", name="warmps")
        for i in range(8):
            nc.tensor.matmul(wps[:], warm[:, 0:128], warm[:],
                             start=True, stop=True)

        # --- sync-queue DMAs: tiny consts, then odd xt chunks, wl0 half,
        #     even wl tensors.  scalar-queue: even xt chunks, wl0 half,
        #     odd wl tensors. ---
        ones = consts.tile([1, 128], f16, tag="ones")
        nc.sync.dma_start(ones[:], ones_d[:, :])
        wd_sb = consts.tile([128, KC * ND], f16, tag="wd")
        nc.sync.dma_start(wd_sb[:], wd_d[:, :])
        bd_sb = consts.tile([1, ND], f16, tag="bd")
        nc.sync.dma_start(bd_sb[:], bd_d[:, :])
        bl_sb = consts.tile([128, H], f16, tag="bl")
        nc.sync.dma_start(bl_sb[:], bl_d[:, :])
        iden = consts.tile([128, 128], f32, tag="iden")
        nc.sync.dma_start(iden[:], iden_d[:, :])

        xt = xt_pool.tile([128, KC * T], f16, tag="xt")
        for k in range(KC):
            eng = nc.scalar if k % 2 == 0 else nc.sync
            eng.dma_start(xt[:, k * T:(k + 1) * T],
                          xt_d[:, k * T:(k + 1) * T])
        wls = []
        for l in range(LPC):
            w = wl_pool.tile([128, KC * H], f16, tag=f"wl{l}", name=f"wl{l}")
            wls.append(w)
        HF = KC * H // 2
        nc.sync.dma_start(wls[0][:, 0:HF], wl_d[0][:, 0:HF])
        nc.scalar.dma_start(wls[0][:, HF:], wl_d[0][:, HF:])
        for l in range(1, LPC):
            eng = nc.sync if l % 2 == 0 else nc.scalar
            eng.dma_start(wls[l][:], wl_d[l])

        def xk(k, t):  # stationary: chunk k, token tile t
            return xt[:, k * T + t * 128:k * T + (t + 1) * 128]

        # bd broadcast to 128 partitions via ones-vector matmul
        bdb = consts.tile([128, ND], f32, tag="bdb")
        bp = ps_pool.tile([128, 512], f32, tag="ps", name="bdps")
        nc.tensor.matmul(bp[:, 0:ND], ones[:], bd_sb[:], start=True, stop=True)
        nc.vector.tensor_copy(bdb[:], bp[:, 0:ND])

        # --- decisions: 8 psum chains, k-outer (paced by xt arrival) ---
        dec_sb = dec_pool.tile([128, NT * ND], f32, tag="dec", bufs=1)
        path = dec_pool.tile([128, NT * 32], f32, tag="path", bufs=1)
        nc.vector.memset(path[:], 0.0)
        pT4 = [dec_pool.tile([128, 128], f16, tag=f"pT4{i}", bufs=1,
                             name=f"pT4{i}") for i in range(2)]
        accs = [acc_pool.tile([128, H], f32, tag=f"acc{t}", name=f"acc{t}")
                for t in range(NT)]

        dpss = [ps_pool.tile([128, 512], f32, tag="ps", name=f"dp{t}")
                for t in range(NT)]
        for k in range(KC):
            for t in range(NT):
                nc.tensor.matmul(dpss[t][:, 0:ND], xk(k, t),
                                 wd_sb[:, k * ND:(k + 1) * ND],
                                 start=(k == 0), stop=(k == KC - 1))

        def sig_path(t):
            # sigmoid(dec + bd) then the 8 path columns for this tile
            tadd = dec_pool.tile([128, ND], f32, tag="tadd", name=f"tadd{t}")
            nc.vector.tensor_tensor(tadd[:], dpss[t][:, 0:ND], bdb[:], op=ADD)
            dsl = dec_sb[:, t * ND:(t + 1) * ND]
            nc.scalar.activation(dsl, tadd[:], SIG)
            # leaf local l = c2*4 + c1*2 + c0
            # cols: 0,1 root(c0); 2+2*c0+c1; 6+2*(2c1+c0)+c2; 14+l
            d1 = dsl[:, 2:6].rearrange("p (i c) -> p i c", c=2)
            d2 = dsl[:, 6:14].rearrange("p (j c) -> p j c", c=2)
            p4 = dec_pool.tile([128, 4], f32, tag="p4", name=f"p4_{t}")
            nc.vector.tensor_tensor(p4[:, 0:2], dsl[:, 0:2], d1[:, :, 0], op=MULT)
            nc.vector.tensor_tensor(p4[:, 2:4], dsl[:, 0:2], d1[:, :, 1], op=MULT)
            p8 = dec_pool.tile([128, 8], f32, tag="p8", name=f"p8_{t}")
            nc.vector.tensor_tensor(p8[:, 0:4], p4[:], d2[:, :, 0], op=MULT)
            nc.vector.tensor_tensor(p8[:, 4:8], p4[:], d2[:, :, 1], op=MULT)
            pt = path[:, t * 32:t * 32 + 8]
            nc.vector.tensor_tensor(pt, p8[:], dsl[:, 14:22], op=MULT)

        for t in range(NT):
            sig_path(t)

        # --- pathT via 2 batched PE transposes (4 tiles each) ---
        for i in range(2):
            tp = ps_pool.tile([128, 512], f32, tag="ps", name=f"tp{i}")
            nc.tensor.transpose(tp[:, 0:128], path[:, i * 128:(i + 1) * 128],
                                iden[:])
            nc.vector.tensor_copy(pT4[i][:], tp[:, 0:128])

        # --- bias init: acc[t] = pathT_t.T @ bl, row-tiled 4 tiles/group ---
        for i in range(2):           # tile group (tiles 4i..4i+3)
            for h in range(2):       # output half
                bps = [ps_pool.tile([128, 512], f32, tag="ps",
                                    name=f"b{i}{h}{j}") for j in range(4)]
                for j in range(4):
                    nc.tensor.matmul(
                        bps[j][:], pT4[i][32 * j:32 * j + 32, :],
                        bl_sb[32 * j:32 * j + 32, h * 512:(h + 1) * 512],
                        start=True, stop=True, tile_position=(32 * j, 0))
                for j in range(4):
                    t = 4 * i + j
                    nc.vector.tensor_copy(accs[t][:, h * 512:(h + 1) * 512],
                                          bps[j][:])

        # --- leaf passes: l outer, t inner, k inner; evict with path col ---
        def evict(t, l, ps_t, half, out=None):
            pcol = path[:, t * 32 + l:t * 32 + l + 1]
            o = half * 512
            dst = accs[t][:, o:o + 512] if out is None else out[:, o:o + 512]
            nc.vector.scalar_tensor_tensor(
                dst, ps_t[:], pcol, accs[t][:, o:o + 512], op0=MULT, op1=ADD)

        for l in range(LPC):
            wl = wls[l]
            last = l == LPC - 1
            for t in range(NT):
                o16 = None
                if last:
                    o16 = out_pool.tile([128, H], f16, tag="o16",
                                        name=f"o16_{t}")
                psl = ps_pool.tile([128, 512], f32, tag="ps",
                                   name=f"pl{l}_{t}")
                psr = ps_pool.tile([128, 512], f32, tag="ps",
                                   name=f"pr{l}_{t}")
                for k in range(KC):
                    lhsT = xk(k, t)
                    nc.tensor.matmul(psl[:], lhsT,
                                     wl[:, k * H:k * H + 512],
                                     start=(k == 0), stop=(k == KC - 1))
                    nc.tensor.matmul(psr[:], lhsT,
                                     wl[:, k * H + 512:(k + 1) * H],
                                     start=(k == 0), stop=(k == KC - 1))
                evict(t, l, psl, 0, out=o16)
                if last:
                    nc.scalar.dma_start(out_d[t * 128:(t + 1) * 128, 0:512],
                                        o16[:, 0:512])
                evict(t, l, psr, 1, out=o16)
                if last:
                    nc.scalar.dma_start(out_d[t * 128:(t + 1) * 128, 512:1024],
                                        o16[:, 512:1024])

    nc.compile()
    _prog_cache["nc"] = nc
    return nc


def _swizzle_kp(a):
    """[K*128, F] -> [128, K*F] fp16, partition-major contiguous."""
    k, f = a.shape[0] // 128, a.shape[1]
    return np.ascontiguousarray(
        a.reshape(k, 128, f).transpose(1, 0, 2).reshape(128, k * f)
    ).astype(np.float16)


def _core_inputs(x, Wd, bd, Wl, bl):
    """Build the 8 per-core input dicts (host-side sharding)."""
    x2 = np.ascontiguousarray(x, dtype=np.float32).reshape(B * S, H)
    Wd = np.asarray(Wd, dtype=np.float32)
    bd = np.asarray(bd, dtype=np.float32)
    Wl = np.ascontiguousarray(Wl, dtype=np.float32)
    bl = np.asarray(bl, dtype=np.float32)

    xts = [_swizzle_kp(np.ascontiguousarray(x2[d * T:(d + 1) * T].T))
           for d in range(DP)]

    # per-subtree decision matrix [H, ND] and bias [ND]
    wd_cs, bd_cs = [], []
    for e in range(EP):
        wd_c = np.zeros((H, ND), dtype=np.float32)
        bd_c = np.zeros((1, ND), dtype=np.float32)
        wd_c[:, 0:2] = Wd[0]                    # root, both choices
        bd_c[0, 0:2] = bd[0]
        for i in range(2):                      # level-1 nodes 1,2
            wd_c[:, 2 + 2 * i:4 + 2 * i] = Wd[1 + i]
            bd_c[0, 2 + 2 * i:4 + 2 * i] = bd[1 + i]
        for j in range(4):                      # level-2 nodes 3..6
            wd_c[:, 6 + 2 * j:8 + 2 * j] = Wd[3 + j]
            bd_c[0, 6 + 2 * j:8 + 2 * j] = bd[3 + j]
        for m in range(8):                      # level-3 nodes 7..14, choice e
            wd_c[:, 14 + m] = Wd[7 + m, :, e]
            bd_c[0, 14 + m] = bd[7 + m, e]
        wd_cs.append(_swizzle_kp(wd_c))
        bd_cs.append(bd_c.astype(np.float16))

    wl_cs, bl_cs = [], []
    for e in range(EP):
        w8 = np.stack([_swizzle_kp(Wl[LPC * e + l]) for l in range(LPC)])
        wl_cs.append(np.ascontiguousarray(w8))
        # bl replicated in 4 row-groups (row-tiled bias matmuls)
        blc = np.zeros((128, H), dtype=np.float32)
        for j in range(4):
            blc[32 * j:32 * j + LPC] = bl[LPC * e:LPC * (e + 1)]
        bl_cs.append(blc.astype(np.float16))

    ones = np.ones((1, 128), dtype=np.float16)
    iden = np.eye(128, dtype=np.float32)

    in_maps = []
    for c in range(8):
        d, e = c // EP, c % EP
        in_maps.append({
            "xt": xts[d],
            "wl": wl_cs[e],
            "wd": wd_cs[e],
            "bd": bd_cs[e],
            "bl": bl_cs[e],
            "ones": ones,
            "iden": iden,
        })
    return in_maps


def kernel(x, Wd, bd, Wl, bl, _want_results=False):
    from concourse import bass_utils

    nc = _build_program()
    in_maps = _core_inputs(x, Wd, bd, Wl, bl)
    res = bass_utils.run_bass_kernel_spmd(nc, in_maps, list(range(8)))

    out = np.empty((DP, T, H), dtype=np.float32)
    for d in range(DP):
        out[d] = (res.results[d * EP]["out"].astype(np.float32)
                  + res.results[d * EP + 1]["out"].astype(np.float32))
    out = out.reshape(B, S, H)
    if _want_results:
        return out, res
    return out


# revision 16
# speedup vs baseline: 1.0637x; 1.0637x over previous
"""Soft decision-tree layer (depth 4, 16 leaves) on 8 trn2 NeuronCores.

Sharding: 4-way data parallel (1024-token groups) x 2-way expert parallel
(8 leaves per core, one level-3 subtree half).  Each core computes, for
its 1024 tokens t and its 8 leaves l:
  partial[t,:] = sum_l path_l(t) * (x[t] @ Wl[l]) + sum_l path_l(t)*bl[l]
Host upcasts + sums the 2 expert partials per token group.

Everything is SBUF-resident (xt 2MB + wl 16MB fp16 + acc 4MB fp32).
GEMM operands are fp16 (216ns per 128x128x512 matmul measured, LDW
hidden); fp32 accumulation in PSUM/SBUF.

Schedule (from trace analysis):
- inputs stream on BOTH hardware DGE queues (sync + scalar engines);
  xt's 8 chunks alternate queues so decisions start ~8.5us and the
  first leaf pass ~13us
- warmup matmuls run on an uninitialized tile (no DMA/memset dep) so
  the PE HAM clock-gate is warm before real work; the PE stream is
  gap-free afterwards, so HAM never re-throttles
- decisions: 8 psum chains, k-outer, paced by xt chunk arrival
- path-weighted bias sum = K=32 PE matmuls of pathT.T @ bl (replaces
  ~90us of DVE tensor ops); pathT via 2 batched PE transposes; the 4
  tiles' bias matmuls are row-tiled (tile_position) so they pack into
  the PE array concurrently
- leaf passes l-outer t-inner, psl/psr 512-col chains, DVE evicts
- output DMA'd as fp16 per half-tile (host upcasts + sums)
"""

import numpy as np

B, S, H = 2, 2048, 1024
DP, EP = 4, 2            # data-parallel x expert-parallel = 8 cores
T = (B * S) // DP        # 1024 tokens per core
LPC = 16 // EP           # 8 leaves per core
NT = T // 128            # 8 token tiles per core
KC = H // 128            # 8 contraction chunks
ND = 24                  # decision columns (22 used + 2 pad)

_prog_cache = {}


def _build_program():
    if "nc" in _prog_cache:
        return _prog_cache["nc"]

    from contextlib import ExitStack
    import concourse.bacc as bacc
    import concourse.tile as tile
    import concourse.mybir as mybir

    f32 = mybir.dt.float32
    f16 = mybir.dt.float16
    MULT = mybir.AluOpType.mult
    ADD = mybir.AluOpType.add
    SIG = mybir.ActivationFunctionType.Sigmoid

    nc = bacc.Bacc("TRN2", target_bir_lowering=False, debug=False, num_devices=8)

    xt_d = nc.dram_tensor("xt", [128, KC * T], f16, kind="ExternalInput").ap()
    wl_d = nc.dram_tensor("wl", [LPC, 128, KC * H], f16, kind="ExternalInput").ap()
    wd_d = nc.dram_tensor("wd", [128, KC * ND], f16, kind="ExternalInput").ap()
    bd_d = nc.dram_tensor("bd", [128, ND], f32, kind="ExternalInput").ap()
    bl_d = nc.dram_tensor("bl", [128, H], f16, kind="ExternalInput").ap()
    iden_d = nc.dram_tensor("iden", [128, 128], f32, kind="ExternalInput").ap()
    out_d = nc.dram_tensor("out", [T, H], f16, kind="ExternalOutput").ap()

    with tile.TileContext(nc) as tc, ExitStack() as ctx:
        consts = ctx.enter_context(tc.tile_pool(name="consts", bufs=1))
        xt_pool = ctx.enter_context(tc.tile_pool(name="xt", bufs=1))
        wl_pool = ctx.enter_context(tc.tile_pool(name="wl", bufs=1))
        acc_pool = ctx.enter_context(tc.tile_pool(name="acc", bufs=1))
        dec_pool = ctx.enter_context(tc.tile_pool(name="dec", bufs=2))
        out_pool = ctx.enter_context(tc.tile_pool(name="o16", bufs=4))
        ps_pool = ctx.enter_context(tc.tile_pool(name="ps", bufs=8, space="PSUM"))

        # --- PE warmup; the memset rides the GpSimd queue (idle early) so
        #     the first matmul lands right after the engine preambles and
        #     covers the DMA-queue ramp (~8.5us) ---
        warm = consts.tile([128, 512], f16, tag="warm")
        nc.gpsimd.memset(warm[:], 0.0)
        wps = ps_pool.tile([128, 512], f32, tag="ps", name="warmps")
        for i in range(8):
            nc.tensor.matmul(wps[:], warm[:, 0:128], warm[:],
                             start=True, stop=True)

        # --- single sync queue (the two HWDGE queues share ~430GB/s, so
        #     splitting buys nothing); order = tiny consts, xt chunks
        #     (paces decisions), bias consts, wl0 chunks (paces leaf 0),
        #     then the remaining leaf weights ---
        wd_sb = consts.tile([128, KC * ND], f16, tag="wd")
        nc.sync.dma_start(wd_sb[:], wd_d[:, :])
        bdb = consts.tile([128, ND], f32, tag="bdb")
        nc.sync.dma_start(bdb[:], bd_d[:, :])

        xt = xt_pool.tile([128, KC * T], f16, tag="xt")
        for k in range(KC):
            nc.sync.dma_start(xt[:, k * T:(k + 1) * T],
                              xt_d[:, k * T:(k + 1) * T])

        bl_sb = consts.tile([128, H], f16, tag="bl")
        nc.sync.dma_start(bl_sb[:], bl_d[:, :])
        iden = consts.tile([128, 128], f32, tag="iden")
        nc.sync.dma_start(iden[:], iden_d[:, :])

        wls = []
        for l in range(LPC):
            w = wl_pool.tile([128, KC * H], f16, tag=f"wl{l}", name=f"wl{l}")
            wls.append(w)
        for k in range(KC):
            nc.sync.dma_start(wls[0][:, k * H:(k + 1) * H],
                              wl_d[0][:, k * H:(k + 1) * H])
        for l in range(1, LPC):
            nc.sync.dma_start(wls[l][:], wl_d[l])

        def xk(k, t):  # stationary: chunk k, token tile t
            return xt[:, k * T + t * 128:k * T + (t + 1) * 128]

        # --- decisions: 7+1 psum chains, k-outer (paced by xt arrival);
        #     filler matmuls on the warmup psum plug the chunk-arrival
        #     gaps so HAM stays at full clock ---
        dec_sb = dec_pool.tile([128, NT * ND], f32, tag="dec", bufs=1)
        path = dec_pool.tile([128, NT * 32], f32, tag="path", bufs=1)
        nc.vector.memset(path[:], 0.0)
        pT4 = [dec_pool.tile([128, 128], f16, tag=f"pT4{i}", bufs=1,
                             name=f"pT4{i}") for i in range(2)]
        accs = [acc_pool.tile([128, H], f32, tag=f"acc{t}", name=f"acc{t}")
                for t in range(NT)]

        dpss = [ps_pool.tile([128, 512], f32, tag="ps", name=f"dp{t}")
                for t in range(7)]
        for k in range(KC):
            for t in range(7):
                nc.tensor.matmul(dpss[t][:, 0:ND], xk(k, t),
                                 wd_sb[:, k * ND:(k + 1) * ND],
                                 start=(k == 0), stop=(k == KC - 1))
            if k < KC - 1:
                nc.tensor.matmul(wps[:], warm[:, 0:128], warm[:],
                                 start=True, stop=True)
        # tile 7's chain after the loop (xt resident by now); its psum
        # recycles the warmup/filler slot, whose uses all precede it
        dp7 = ps_pool.tile([128, 512], f32, tag="ps", name="dp7")
        for k in range(KC):
            nc.tensor.matmul(dp7[:, 0:ND], xk(k, 7),
                             wd_sb[:, k * ND:(k + 1) * ND],
                             start=(k == 0), stop=(k == KC - 1))
        dpss.append(dp7)

        def sig_path(t):
            # sigmoid(dec + bd) then the 8 path columns for this tile
            tadd = dec_pool.tile([128, ND], f32, tag="tadd", name=f"tadd{t}")
            nc.vector.tensor_tensor(tadd[:], dpss[t][:, 0:ND], bdb[:], op=ADD)
            dsl = dec_sb[:, t * ND:(t + 1) * ND]
            nc.scalar.activation(dsl, tadd[:], SIG)
            # leaf local l = c2*4 + c1*2 + c0
            # cols: 0,1 root(c0); 2+2*c0+c1; 6+2*(2c1+c0)+c2; 14+l
            d1 = dsl[:, 2:6].rearrange("p (i c) -> p i c", c=2)
            d2 = dsl[:, 6:14].rearrange("p (j c) -> p j c", c=2)
            p4 = dec_pool.tile([128, 4], f32, tag="p4", name=f"p4_{t}")
            nc.vector.tensor_tensor(p4[:, 0:2], dsl[:, 0:2], d1[:, :, 0], op=MULT)
            nc.vector.tensor_tensor(p4[:, 2:4], dsl[:, 0:2], d1[:, :, 1], op=MULT)
            p8 = dec_pool.tile([128, 8], f32, tag="p8", name=f"p8_{t}")
            nc.vector.tensor_tensor(p8[:, 0:4], p4[:], d2[:, :, 0], op=MULT)
            nc.vector.tensor_tensor(p8[:, 4:8], p4[:], d2[:, :, 1], op=MULT)
            pt = path[:, t * 32:t * 32 + 8]
            nc.vector.tensor_tensor(pt, p8[:], dsl[:, 14:22], op=MULT)

        for t in range(NT):
            sig_path(t)

        # --- pathT via 2 batched PE transposes (4 tiles each) ---
        for i in range(2):
            tp = ps_pool.tile([128, 512], f32, tag="ps", name=f"tp{i}")
            nc.tensor.transpose(tp[:, 0:128], path[:, i * 128:(i + 1) * 128],
                                iden[:])
            nc.vector.tensor_copy(pT4[i][:], tp[:, 0:128])

        # --- bias init: acc[t] = pathT_t.T @ bl, row-tiled 4 tiles/group ---
        for i in range(2):           # tile group (tiles 4i..4i+3)
            for h in range(2):       # output half
                bps = [ps_pool.tile([128, 512], f32, tag="ps",
                                    name=f"b{i}{h}{j}") for j in range(4)]
                for j in range(4):
                    nc.tensor.matmul(
                        bps[j][:], pT4[i][32 * j:32 * j + 32, :],
                        bl_sb[32 * j:32 * j + 32, h * 512:(h + 1) * 512],
                        start=True, stop=True, tile_position=(32 * j, 0))
                for j in range(4):
                    t = 4 * i + j
                    nc.vector.tensor_copy(accs[t][:, h * 512:(h + 1) * 512],
                                          bps[j][:])

        # --- leaf passes: l outer, t inner, k inner; evict with path col ---
        def evict(t, l, ps_t, half, out=None):
            pcol = path[:, t * 32 + l:t * 32 + l + 1]
            o = half * 512
            dst = accs[t][:, o:o + 512] if out is None else out[:, o:o + 512]
            nc.vector.scalar_tensor_tensor(
                dst, ps_t[:], pcol, accs[t][:, o:o + 512], op0=MULT, op1=ADD)

        for l in range(LPC):
            wl = wls[l]
            last = l == LPC - 1
            for t in range(NT):
                o16 = None
                if last:
                    o16 = out_pool.tile([128, H], f16, tag="o16",
                                        name=f"o16_{t}")
                psl = ps_pool.tile([128, 512], f32, tag="ps",
                                   name=f"pl{l}_{t}")
                psr = ps_pool.tile([128, 512], f32, tag="ps",
                                   name=f"pr{l}_{t}")
                for k in range(KC):
                    lhsT = xk(k, t)
                    nc.tensor.matmul(psl[:], lhsT,
                                     wl[:, k * H:k * H + 512],
                                     start=(k == 0), stop=(k == KC - 1))
                    nc.tensor.matmul(psr[:], lhsT,
                                     wl[:, k * H + 512:(k + 1) * H],
                                     start=(k == 0), stop=(k == KC - 1))
                evict(t, l, psl, 0, out=o16)
                if last:
                    nc.scalar.dma_start(out_d[t * 128:(t + 1) * 128, 0:512],
                                        o16[:, 0:512])
                evict(t, l, psr, 1, out=o16)
                if last:
                    nc.scalar.dma_start(out_d[t * 128:(t + 1) * 128, 512:1024],
                                        o16[:, 512:1024])

    nc.compile()
    _prog_cache["nc"] = nc
    return nc


def _swizzle_kp(a):
    """[K*128, F] -> [128, K*F] fp16, partition-major contiguous."""
    k, f = a.shape[0] // 128, a.shape[1]
    return np.ascontiguousarray(
        a.reshape(k, 128, f).transpose(1, 0, 2).reshape(128, k * f)
    ).astype(np.float16)


def _core_inputs(x, Wd, bd, Wl, bl):
    """Build the 8 per-core input dicts (host-side sharding)."""
    x2 = np.ascontiguousarray(x, dtype=np.float32).reshape(B * S, H)
    Wd = np.asarray(Wd, dtype=np.float32)
    bd = np.asarray(bd, dtype=np.float32)
    Wl = np.ascontiguousarray(Wl, dtype=np.float32)
    bl = np.asarray(bl, dtype=np.float32)

    xts = [_swizzle_kp(np.ascontiguousarray(x2[d * T:(d + 1) * T].T))
           for d in range(DP)]

    # per-subtree decision matrix [H, ND] and bias [ND]
    wd_cs, bd_cs = [], []
    for e in range(EP):
        wd_c = np.zeros((H, ND), dtype=np.float32)
        bd_c = np.zeros((1, ND), dtype=np.float32)
        wd_c[:, 0:2] = Wd[0]                    # root, both choices
        bd_c[0, 0:2] = bd[0]
        for i in range(2):                      # level-1 nodes 1,2
            wd_c[:, 2 + 2 * i:4 + 2 * i] = Wd[1 + i]
            bd_c[0, 2 + 2 * i:4 + 2 * i] = bd[1 + i]
        for j in range(4):                      # level-2 nodes 3..6
            wd_c[:, 6 + 2 * j:8 + 2 * j] = Wd[3 + j]
            bd_c[0, 6 + 2 * j:8 + 2 * j] = bd[3 + j]
        for m in range(8):                      # level-3 nodes 7..14, choice e
            wd_c[:, 14 + m] = Wd[7 + m, :, e]
            bd_c[0, 14 + m] = bd[7 + m, e]
        wd_cs.append(_swizzle_kp(wd_c))
        bd_cs.append(np.ascontiguousarray(
            np.broadcast_to(bd_c, (128, ND))).astype(np.float32))

    wl_cs, bl_cs = [], []
    for e in range(EP):
        w8 = np.stack([_swizzle_kp(Wl[LPC * e + l]) for l in range(LPC)])
        wl_cs.append(np.ascontiguousarray(w8))
        # bl replicated in 4 row-groups (row-tiled bias matmuls)
        blc = np.zeros((128, H), dtype=np.float32)
        for j in range(4):
            blc[32 * j:32 * j + LPC] = bl[LPC * e:LPC * (e + 1)]
        bl_cs.append(blc.astype(np.float16))

    iden = np.eye(128, dtype=np.float32)

    in_maps = []
    for c in range(8):
        d, e = c // EP, c % EP
        in_maps.append({
            "xt": xts[d],
            "wl": wl_cs[e],
            "wd": wd_cs[e],
            "bd": bd_cs[e],
            "bl": bl_cs[e],
            "iden": iden,
        })
    return in_maps


def kernel(x, Wd, bd, Wl, bl, _want_results=False):
    from concourse import bass_utils

    nc = _build_program()
    in_maps = _core_inputs(x, Wd, bd, Wl, bl)
    res = bass_utils.run_bass_kernel_spmd(nc, in_maps, list(range(8)))

    out = np.empty((DP, T, H), dtype=np.float32)
    for d in range(DP):
        out[d] = (res.results[d * EP]["out"].astype(np.float32)
                  + res.results[d * EP + 1]["out"].astype(np.float32))
    out = out.reshape(B, S, H)
    if _want_results:
        return out, res
    return out
